# revision 21
# baseline (speedup 1.0000x reference)
"""BiMamba (fwd+bwd Mamba + merge) Trainium2 Bass kernel — v2.

Sharding (8 cores): core = batch*4 + dir*2 + e_half.
Each core computes one (batch, direction) pair over 1024 of the 2048 d_inner
channels, in e-partition layout [e_p=128 x 8 tiles, t_free=1024].
bwd cores operate entirely in flipped time (host pre-flips x); the final
out_proj partial is un-flipped via a data-driven mask combine, then a 4-core
ReduceScatter produces d-sharded output per batch group.

v2 structure (from trace analysis of v1 @774us):
 - Phase A fuses in_proj + conv + silu + x_proj PSUM accumulation; one f16
   AllReduce for all 96 x_proj rows (v1 had two).
 - softplus split into an Exp burst then an Ln burst across m-tiles so the
   scalar engine loads each activation table twice total instead of 2x/m.
 - Scan groups pack 4 n-planes per tensor_tensor_scan (dA stays f32: scan
   throughput measured dtype-invariant at ~2.58ns/elem).
 - y = sum_n C_n*h_n accumulated on the tensor engine via identity matmuls
   into PSUM (replaces v1's DVE add tree); dp*xc skip rides a diag matmul.
 - Phase D mask-combine uses two PSUM-source tensor_scalar ops + one add.

Self-contained: hardcodes B=2, L=1024, D=1024, E=2048 (1024/core), N=16,
dt_rank=64, d_conv=4.
"""
import numpy as np

B, L, D = 2, 1024, 1024
E = 2048
EH = 1024            # channels per core (half of E)
N = 16
DTR = 64
K = 4                # d_conv
M_TILES = 8          # e-tiles per core
NPB = 4              # n-planes per scan group
NG = N // NPB        # scan groups per m-tile
PL = L + 2           # plane stride with 2-col zero gap for the packed scan

_nc_cache = {}


def _build_nc():
    import concourse.bacc as bacc
    import concourse.mybir as mybir
    from concourse import tile

    f32, f16 = mybir.dt.float32, mybir.dt.float16
    Alu = mybir.AluOpType
    Act = mybir.ActivationFunctionType

    nc = bacc.Bacc("TRN2", target_bir_lowering=False, debug=False, num_devices=8)

    # ---- DRAM I/O ----
    xT_d = nc.dram_tensor("xT", [D, 3 + L], f16, kind="ExternalInput")
    # pre-tiled: [p, m*1024 + kt*128 + e']  (one DMA per m-slab)
    wxiT_d = nc.dram_tensor("wxiT", [128, M_TILES * EH], f16, kind="ExternalInput")
    wzT_d = nc.dram_tensor("wzT", [128, M_TILES * EH], f16, kind="ExternalInput")
    convw_d = nc.dram_tensor("convw", [128, M_TILES * K], f32, kind="ExternalInput")
    convb_d = nc.dram_tensor("convb", [128, M_TILES], f32, kind="ExternalInput")
    xpT_d = nc.dram_tensor("xpT", [EH, 96], f16, kind="ExternalInput")
    dtwT_d = nc.dram_tensor("dtwT", [DTR, EH], f16, kind="ExternalInput")
    dtb_d = nc.dram_tensor("dtb", [128, M_TILES], f32, kind="ExternalInput")
    arate_d = nc.dram_tensor("arate", [128, M_TILES * N], f32, kind="ExternalInput")
    ident_d = nc.dram_tensor("ident", [128, 128], f16, kind="ExternalInput")
    diagdp_d = nc.dram_tensor("diagdp", [128, M_TILES * 128], f16, kind="ExternalInput")
    # pre-tiled: [p, dm*1024 + m*128 + d']
    woT_d = nc.dram_tensor("woT", [128, M_TILES * D], f16, kind="ExternalInput")
    mf_d = nc.dram_tensor("mf", [128, 1], f32, kind="ExternalInput")
    mb_d = nc.dram_tensor("mb", [128, 1], f32, kind="ExternalInput")

    ar_in = nc.dram_tensor("ar_in", [96, L], f16, kind="Internal")
    ar_out1 = nc.dram_tensor("ar_out1", [DTR, L], f16, kind="Internal")
    ar_out2 = nc.dram_tensor("ar_out2", [32, L], f16, kind="Internal")
    oc_in = nc.dram_tensor("oc_in", [D, L], f16, kind="Internal")
    oc_out = nc.dram_tensor("oc_out", [256, L], f16, kind="Internal")
    out_d = nc.dram_tensor("out_p", [256, L], f16, kind="ExternalOutput")

    dma_rr = None

    with tile.TileContext(nc) as tc:
        with tc.tile_pool(name="const", bufs=1) as cpool, \
             tc.tile_pool(name="res", bufs=1) as rpool:
            convw = cpool.tile([128, M_TILES * K], f32)
            convb = cpool.tile([128, M_TILES], f32)
            dtb = cpool.tile([128, M_TILES], f32)
            arate = cpool.tile([128, M_TILES * N], f32)
            ident = cpool.tile([128, 128], f16)
            diagdp = cpool.tile([128, M_TILES * 128], f16)
            xpall = cpool.tile([128, M_TILES * 96], f16)
            dtwall = cpool.tile([DTR, M_TILES * 128], f16)
            mf = cpool.tile([128, 1], f32)
            mb = cpool.tile([128, 1], f32)
            engs = [nc.sync, nc.gpsimd, nc.scalar]

            xc16 = rpool.tile([128, M_TILES * L], f16)
            sz16 = rpool.tile([128, M_TILES * L], f16)
            g16 = rpool.tile([128, M_TILES * L], f16)
            delta16 = rpool.tile([128, M_TILES * L], f16)
            bca = rpool.tile([128, N * L], f16)
            bcc = rpool.tile([128, N * L], f16)
            dtrows = rpool.tile([DTR, L], f16)

            # ---------- Phase A: in_proj + conv + silu + x_proj accum ----------
            with tc.tile_pool(name="pa", bufs=1) as pap, \
                 tc.tile_pool(name="paw", bufs=4) as pwp, \
                 tc.tile_pool(name="pcv", bufs=2) as pcv, \
                 tc.tile_pool(name="psA1", bufs=2, space="PSUM") as psA1, \
                 tc.tile_pool(name="psA2", bufs=1, space="PSUM") as psA2, \
                 tc.tile_pool(name="psB", bufs=1, space="PSUM") as psB:
                xT = pap.tile([128, M_TILES * (3 + L)], f16)
                # single strided DMA for the whole input (keeps sync/scalar
                # queues free so m0's weights land first)
                nc.gpsimd.dma_start(
                    xT[:].rearrange("p (k c) -> p k c", c=3 + L),
                    xT_d[:].rearrange("(k p) c -> p k c", p=128))
                # consts are consumed late; issue them behind the hot DMAs
                nc.sync.dma_start(
                    xpall[:].rearrange("p (m f) -> p m f", f=96),
                    xpT_d[:].rearrange("(m p) f -> p m f", p=128))
                for i, (t_, d_) in enumerate((
                        (convw, convw_d), (convb, convb_d), (dtb, dtb_d),
                        (arate, arate_d), (ident, ident_d), (diagdp, diagdp_d),
                        (dtwall, dtwT_d), (mf, mf_d), (mb, mb_d))):
                    engs[i % 3].dma_start(t_[:], d_[:])
                ps_dbl = psB.tile([96, L], f32)
                for m in range(M_TILES):
                    wxi = pwp.tile([128, EH], f16, tag="wxi")
                    wz = pwp.tile([128, EH], f16, tag="wz")
                    nc.sync.dma_start(wxi[:, 0:512], wxiT_d[:, m * EH:m * EH + 512])
                    nc.gpsimd.dma_start(wxi[:, 512:EH], wxiT_d[:, m * EH + 512:(m + 1) * EH])
                    nc.scalar.dma_start(wz[:, 0:512], wzT_d[:, m * EH:m * EH + 512])
                    nc.sync.dma_start(wz[:, 512:EH], wzT_d[:, m * EH + 512:(m + 1) * EH])
                    ps_xi = psA1.tile([128, L], f32, tag="xi")
                    ps_z = psA2.tile([128, L], f32, tag="z")
                    for kt in range(M_TILES):
                        xk = xT[:, kt * (3 + L):(kt + 1) * (3 + L)]
                        for h in range(2):
                            nc.tensor.matmul(ps_xi[:, h * 512:(h + 1) * 512],
                                             wxi[:, kt * 128:(kt + 1) * 128],
                                             xk[:, 3 + h * 512: 3 + (h + 1) * 512],
                                             start=(kt == 0), stop=(kt == M_TILES - 1))
                    for kt in range(M_TILES):
                        xk = xT[:, kt * (3 + L):(kt + 1) * (3 + L)]
                        for h in range(2):
                            nc.tensor.matmul(ps_z[:, h * 512:(h + 1) * 512],
                                             wz[:, kt * 128:(kt + 1) * 128],
                                             xk[:, 3 + h * 512: 3 + (h + 1) * 512],
                                             start=(kt == 0), stop=(kt == M_TILES - 1))
                    # conv: f16 padded copy, 4 taps as tensor_scalar muls + adds
                    xi16 = pcv.tile([128, 3 + L], f16, tag="xi16")
                    if m < 2:
                        nc.vector.memset(xi16[:, 0:3], 0.0)
                    nc.vector.tensor_copy(xi16[:, 3:3 + L], ps_xi[:])
                    tp = pcv.tile([128, 4 * L], f16, tag="taps")
                    for k in range(K):
                        nc.vector.tensor_scalar_mul(tp[:, k * L:(k + 1) * L],
                                                    xi16[:, k:k + L],
                                                    convw[:, m * K + k:m * K + k + 1])
                    t01 = pcv.tile([128, L], f16, tag="t01")
                    t23 = pcv.tile([128, L], f16, tag="t23")
                    cacc = pcv.tile([128, L], f16, tag="cacc")
                    nc.vector.tensor_add(t01[:], tp[:, 0:L], tp[:, L:2 * L])
                    nc.vector.tensor_add(t23[:], tp[:, 2 * L:3 * L], tp[:, 3 * L:4 * L])
                    nc.vector.tensor_add(cacc[:], t01[:], t23[:])
                    nc.scalar.activation(xc16[:, m * L:(m + 1) * L], cacc[:],
                                         Act.Silu, bias=convb[:, m:m + 1])
                    nc.scalar.activation(sz16[:, m * L:(m + 1) * L], ps_z[:], Act.Silu)
                    for h in range(2):
                        nc.tensor.matmul(ps_dbl[:, h * 512:(h + 1) * 512],
                                         xpall[:, m * 96:(m + 1) * 96],
                                         xc16[:, m * L + h * 512: m * L + (h + 1) * 512],
                                         start=(m == 0), stop=(m == M_TILES - 1))
                dbl16 = pap.tile([96, L], f16)
                nc.vector.tensor_copy(dbl16[:], ps_dbl[:])
                nc.sync.dma_start(ar_in[:], dbl16[:])
                # dt rows first so softplus can start while B/C rows reduce
                nc.gpsimd.collective_compute(
                    "AllReduce", Alu.add,
                    replica_groups=[[0, 1], [2, 3], [4, 5], [6, 7]],
                    ins=[ar_in[0:DTR, :]], outs=[ar_out1[:]])
                nc.gpsimd.collective_compute(
                    "AllReduce", Alu.add,
                    replica_groups=[[0, 1], [2, 3], [4, 5], [6, 7]],
                    ins=[ar_in[DTR:96, :]], outs=[ar_out2[:]])
                nc.sync.dma_start(dtrows[:], ar_out1[:])
                nc.sync.dma_start(
                    bca[:].rearrange("p (n l) -> p n l", l=L),
                    ar_out2[None, 0:N, :].broadcast_to([128, N, L]))
                nc.gpsimd.dma_start(
                    bcc[:].rearrange("p (n l) -> p n l", l=L),
                    ar_out2[None, N:2 * N, :].broadcast_to([128, N, L]))

            # ---------- Phase C: softplus (bursts of 3), dA planes, scan ----------
            with tc.tile_pool(name="pee", bufs=3) as pee, \
                 tc.tile_pool(name="pc", bufs=2) as pcp, \
                 tc.tile_pool(name="psY", bufs=2, space="PSUM") as psY:
                bca3 = bca[:].rearrange("p (n l) -> p n l", l=L)
                bcc3 = bcc[:].rearrange("p (n l) -> p n l", l=L)

                def cmain(m):
                    u16 = pcp.tile([128, L], f16, tag="u16")
                    nc.vector.tensor_mul(u16[:], delta16[:, m * L:(m + 1) * L],
                                         xc16[:, m * L:(m + 1) * L])
                    ps_y = psY.tile([128, L], f32, tag="y")
                    for g in range(NG):
                        dA = pcp.tile([128, NPB * PL], f16, tag="dA")
                        for j in range(NPB):
                            n = g * NPB + j
                            nc.scalar.activation(dA[:, j * PL:j * PL + L],
                                                 delta16[:, m * L:(m + 1) * L],
                                                 Act.Exp,
                                                 scale=arate[:, m * N + n:m * N + n + 1])
                        dA3 = dA[:].rearrange("p (n l) -> p n l", l=PL)
                        dBu = pcp.tile([128, NPB * PL], f16, tag="dBu")
                        dBu3 = dBu[:].rearrange("p (n l) -> p n l", l=PL)
                        if m == 0 and g < 2:
                            # gap columns stay 0 across slot reuse (2 slots/tag)
                            nc.vector.memset(dA3[:, :, L:PL], 0.0)
                            nc.vector.memset(dBu3[:, :, L:PL], 0.0)
                        nc.vector.tensor_mul(
                            dBu3[:, :, 0:L],
                            u16[:, None, :].broadcast_to([128, NPB, L]),
                            bca3[:, g * NPB:(g + 1) * NPB, :])
                        h4 = pcp.tile([128, NPB * PL], f16, tag="h4")
                        nc.vector.tensor_tensor_scan(h4[:], dA[:], dBu[:], 0.0,
                                                     Alu.mult, Alu.add)
                        h43 = h4[:].rearrange("p (n l) -> p n l", l=PL)
                        prod = pcp.tile([128, NPB * PL], f16, tag="dBu")
                        prod3 = prod[:].rearrange("p (n l) -> p n l", l=PL)
                        nc.vector.tensor_mul(prod3[:, :, 0:L], h43[:, :, 0:L],
                                             bcc3[:, g * NPB:(g + 1) * NPB, :])
                        for j in range(NPB):
                            for h in range(2):
                                nc.tensor.matmul(
                                    ps_y[:, h * 512:(h + 1) * 512], ident[:],
                                    prod[:, j * PL + h * 512: j * PL + h * 512 + 512],
                                    start=(g == 0 and j == 0), stop=False)
                    # dp * xc skip-connection rides a diagonal matmul
                    for h in range(2):
                        nc.tensor.matmul(ps_y[:, h * 512:(h + 1) * 512],
                                         diagdp[:, m * 128:(m + 1) * 128],
                                         xc16[:, m * L + h * 512: m * L + (h + 1) * 512],
                                         start=False, stop=(h == 1))
                    y16s = pcp.tile([128, L], f16, tag="y16s")
                    nc.scalar.activation(y16s[:], ps_y[:], Act.Copy)
                    nc.vector.tensor_mul(g16[:, m * L:(m + 1) * L], y16s[:],
                                         sz16[:, m * L:(m + 1) * L])

                with tc.tile_pool(name="psP", bufs=2, space="PSUM") as psP:
                    # softplus in bursts of 3 m-tiles (Exp x3 then Ln x3) so the
                    # scalar engine switches act tables ~6x total, and C-main
                    # m=0 unblocks after the first burst.
                    for m0 in range(0, M_TILES, 3):
                        ms = range(m0, min(m0 + 3, M_TILES))
                        ees = {}
                        for m in ms:
                            ps_dt = psP.tile([128, L], f32, tag="dt")
                            for h in range(2):
                                nc.tensor.matmul(ps_dt[:, h * 512:(h + 1) * 512],
                                                 dtwall[:, m * 128:(m + 1) * 128],
                                                 dtrows[:, h * 512:(h + 1) * 512],
                                                 start=True, stop=True)
                            ee = pee.tile([128, L], f32, tag="ee")
                            nc.scalar.activation(ee[:], ps_dt[:],
                                                 Act.Exp, bias=dtb[:, m:m + 1])
                            ees[m] = ee
                        for m in ms:
                            nc.scalar.activation(delta16[:, m * L:(m + 1) * L],
                                                 ees[m][:], Act.Ln, bias=1.0)
                    for m in range(M_TILES):
                        cmain(m)

            # ---------- Phase D: out_proj + flip-combine + ReduceScatter ----------
            with tc.tile_pool(name="pd", bufs=2) as pdp, \
                 tc.tile_pool(name="psD", bufs=2, space="PSUM") as psD:
                for dm in range(M_TILES):
                    wo = pdp.tile([128, D], f16, tag="wo")
                    nc.sync.dma_start(wo[:, 0:512], woT_d[:, dm * D:dm * D + 512])
                    nc.gpsimd.dma_start(wo[:, 512:D], woT_d[:, dm * D + 512:(dm + 1) * D])
                    ps_o = psD.tile([128, L], f32, tag="o")
                    for m in range(M_TILES):
                        for h in range(2):
                            nc.tensor.matmul(ps_o[:, h * 512:(h + 1) * 512],
                                             wo[:, m * 128:(m + 1) * 128],
                                             g16[:, m * L + h * 512: m * L + (h + 1) * 512],
                                             start=(m == 0), stop=(m == M_TILES - 1))
                    t1 = pdp.tile([128, L], f16, tag="t1")
                    r1 = pdp.tile([128, L], f16, tag="r1")
                    ocs = pdp.tile([128, L], f16, tag="ocs")
                    nc.vector.tensor_scalar_mul(t1[:], ps_o[:], mf[:, 0:1])
                    nc.vector.tensor_scalar_mul(r1[:], ps_o[:, ::-1], mb[:, 0:1])
                    nc.vector.tensor_add(ocs[:], t1[:], r1[:])
                    nc.sync.dma_start(oc_in[dm * 128:(dm + 1) * 128, :], ocs[:])
                    if dm % 2 == 1:
                        # ReduceScatter: group-rank ci gets a contiguous 64-row
                        # shard of each 256-row chunk; host stitches shards.
                        ch = dm // 2
                        nc.gpsimd.collective_compute(
                            "ReduceScatter", Alu.add,
                            replica_groups=[[0, 1, 2, 3], [4, 5, 6, 7]],
                            ins=[oc_in[ch * 256:(ch + 1) * 256, :]],
                            outs=[oc_out[ch * 64:(ch + 1) * 64, :]])
                        nc.sync.dma_start(out_d[ch * 64:(ch + 1) * 64, :],
                                          oc_out[ch * 64:(ch + 1) * 64, :])

    nc.compile()
    return nc


def _host_prep(inputs):
    """Build the 8 per-core input maps from the full problem inputs."""
    x = np.asarray(inputs["x"], np.float32)
    merge_w = np.asarray(inputs["merge_w"], np.float32)
    in_maps = []
    for b in range(B):
        for di, pre in enumerate(("fwd", "bwd")):
            p = {k: np.asarray(inputs[f"{pre}_{k}"], np.float32)
                 for k in ("in_proj", "conv_w", "conv_b", "x_proj", "dt_w",
                           "dt_b", "A_log", "D", "out_proj")}
            xb = x[b]
            if di == 1:
                xb = xb[::-1]
            xTp = np.concatenate([np.zeros((D, 3), np.float32), xb.T], axis=1)
            A = -np.exp(p["A_log"])                       # (E, N)
            W = merge_w[:, di * D:(di + 1) * D] @ p["out_proj"]   # (D, E)
            def pack_lhsT(wT):
                # (D, EH) -> [p, m*1024 + kt*128 + e']
                return np.ascontiguousarray(
                    wT.reshape(M_TILES, 128, M_TILES, 128).transpose(1, 2, 0, 3)
                    .reshape(128, M_TILES * EH))

            for half in range(2):
                sl = slice(half * EH, (half + 1) * EH)
                wxiT = pack_lhsT(p["in_proj"][:E][sl].T)
                wzT = pack_lhsT(p["in_proj"][E:][sl].T)
                convw = p["conv_w"][sl].reshape(M_TILES, 128, K).transpose(1, 0, 2).reshape(128, M_TILES * K)
                convb = p["conv_b"][sl].reshape(M_TILES, 128).T
                xpT = p["x_proj"][:, sl].T                # (EH, 96)
                dtwT = p["dt_w"][sl].T                    # (DTR, EH)
                dtb = p["dt_b"][sl].reshape(M_TILES, 128).T
                arate = A[sl].reshape(M_TILES, 128, N).transpose(1, 0, 2).reshape(128, M_TILES * N)
                dp128 = p["D"][sl].reshape(M_TILES, 128).T    # [128, M]
                diagdp = np.zeros((128, M_TILES * 128), np.float32)
                for m in range(M_TILES):
                    diagdp[np.arange(128), m * 128 + np.arange(128)] = dp128[:, m]
                woT = pack_lhsT(W[:, sl].T)               # (EH, D) pre-tiled
                fwd = (di == 0)
                in_maps.append({
                    "xT": xTp.astype(np.float16),
                    "wxiT": wxiT.astype(np.float16),
                    "wzT": wzT.astype(np.float16),
                    "convw": np.ascontiguousarray(convw, np.float32),
                    "convb": np.ascontiguousarray(convb, np.float32),
                    "xpT": xpT.astype(np.float16),
                    "dtwT": np.ascontiguousarray(dtwT).astype(np.float16),
                    "dtb": np.ascontiguousarray(dtb, np.float32),
                    "arate": np.ascontiguousarray(arate, np.float32),
                    "ident": np.eye(128, dtype=np.float16),
                    "diagdp": diagdp.astype(np.float16),
                    "woT": woT.astype(np.float16),
                    "mf": np.full((128, 1), 1.0 if fwd else 0.0, np.float32),
                    "mb": np.full((128, 1), 0.0 if fwd else 1.0, np.float32),
                })
    return in_maps


def _ensure_neuron_platform():
    """If a caller pinned jax to cpu, re-point it at the neuron/axon PJRT
    platform so run_bass_kernel_spmd sees the 8 NeuronCores."""
    import jax
    try:
        if len(jax.devices()) >= 8 and jax.devices()[0].platform != "cpu":
            return
    except Exception:
        pass
    for plat in ("axon", "neuron"):
        try:
            jax.config.update("jax_platforms", plat)
            if len(jax.devices()) >= 8:
                return
        except Exception:
            continue


def kernel(**inputs):
    _ensure_neuron_platform()
    from concourse.bass_utils import run_bass_kernel_spmd
    if "nc" not in _nc_cache:
        _nc_cache["nc"] = _build_nc()
    nc = _nc_cache["nc"]
    in_maps = _host_prep(inputs)
    res = run_bass_kernel_spmd(nc, in_maps, core_ids=list(range(8)))
    _nc_cache["last_results"] = res
    # Stitch ReduceScatter shards: 4 chunks of 256 d-rows; within chunk ch,
    # group-rank ci holds rows [ch*256 + ci*64 : +64] at out_p[ch*64:(ch+1)*64].
    out = np.zeros((B, L, D), np.float32)
    for b in range(B):
        od = np.zeros((D, L), np.float32)
        for ci in range(4):
            shard = res.results[4 * b + ci]["out_p"].astype(np.float32)
            for ch in range(4):
                od[ch * 256 + ci * 64: ch * 256 + (ci + 1) * 64] = \
                    shard[ch * 64:(ch + 1) * 64]
        out[b] = od.T
    return out


# revision 22
# speedup vs baseline: 1.1466x; 1.1466x over previous
"""BiMamba (fwd+bwd Mamba + merge) Trainium2 Bass kernel — v2.

Sharding (8 cores): core = batch*4 + dir*2 + e_half.
Each core computes one (batch, direction) pair over 1024 of the 2048 d_inner
channels, in e-partition layout [e_p=128 x 8 tiles, t_free=1024].
bwd cores operate entirely in flipped time (host pre-flips x); the final
out_proj partial is un-flipped via a data-driven mask combine, then a 4-core
ReduceScatter produces d-sharded output per batch group.

v2 structure (from trace analysis of v1 @774us):
 - Phase A fuses in_proj + conv + silu + x_proj PSUM accumulation; one f16
   AllReduce for all 96 x_proj rows (v1 had two).
 - softplus split into an Exp burst then an Ln burst across m-tiles so the
   scalar engine loads each activation table twice total instead of 2x/m.
 - Scan groups pack 4 n-planes per tensor_tensor_scan (dA stays f32: scan
   throughput measured dtype-invariant at ~2.58ns/elem).
 - y = sum_n C_n*h_n accumulated on the tensor engine via identity matmuls
   into PSUM (replaces v1's DVE add tree); dp*xc skip rides a diag matmul.
 - Phase D mask-combine uses two PSUM-source tensor_scalar ops + one add.

Self-contained: hardcodes B=2, L=1024, D=1024, E=2048 (1024/core), N=16,
dt_rank=64, d_conv=4.
"""
import numpy as np

B, L, D = 2, 1024, 1024
E = 2048
EH = 1024            # channels per core (half of E)
N = 16
DTR = 64
K = 4                # d_conv
M_TILES = 8          # e-tiles per core
NPB = 4              # n-planes per scan group
NG = N // NPB        # scan groups per m-tile
PL = L + 2           # plane stride with 2-col zero gap for the packed scan

_nc_cache = {}


def _build_nc():
    import concourse.bacc as bacc
    import concourse.mybir as mybir
    from concourse import tile

    f32, f16 = mybir.dt.float32, mybir.dt.float16
    Alu = mybir.AluOpType
    Act = mybir.ActivationFunctionType

    nc = bacc.Bacc("TRN2", target_bir_lowering=False, debug=False, num_devices=8)

    # ---- DRAM I/O ----
    xT_d = nc.dram_tensor("xT", [D, 3 + L], f16, kind="ExternalInput")
    # pre-tiled: [p, m*1024 + kt*128 + e']  (one DMA per m-slab)
    wxiT_d = nc.dram_tensor("wxiT", [128, M_TILES * EH], f16, kind="ExternalInput")
    wzT_d = nc.dram_tensor("wzT", [128, M_TILES * EH], f16, kind="ExternalInput")
    convw_d = nc.dram_tensor("convw", [128, M_TILES * K], f32, kind="ExternalInput")
    convb_d = nc.dram_tensor("convb", [128, M_TILES], f32, kind="ExternalInput")
    xpT_d = nc.dram_tensor("xpT", [EH, 96], f16, kind="ExternalInput")
    dtwT_d = nc.dram_tensor("dtwT", [DTR, EH], f16, kind="ExternalInput")
    dtb_d = nc.dram_tensor("dtb", [128, M_TILES], f32, kind="ExternalInput")
    arate_d = nc.dram_tensor("arate", [128, M_TILES * N], f32, kind="ExternalInput")
    ident_d = nc.dram_tensor("ident", [128, 128], f16, kind="ExternalInput")
    diagdp_d = nc.dram_tensor("diagdp", [128, M_TILES * 128], f16, kind="ExternalInput")
    # pre-tiled: [p, dm*1024 + m*128 + d']
    woT_d = nc.dram_tensor("woT", [128, M_TILES * D], f16, kind="ExternalInput")
    mf_d = nc.dram_tensor("mf", [128, 1], f32, kind="ExternalInput")
    mb_d = nc.dram_tensor("mb", [128, 1], f32, kind="ExternalInput")

    ar_in = nc.dram_tensor("ar_in", [96, L], f16, kind="Internal")
    ar_out1 = nc.dram_tensor("ar_out1", [DTR, L], f16, kind="Internal")
    ar_out2 = nc.dram_tensor("ar_out2", [32, L], f16, kind="Internal")
    oc_in = nc.dram_tensor("oc_in", [D, L], f16, kind="Internal")
    oc_out = nc.dram_tensor("oc_out", [256, L], f16, kind="Internal")
    out_d = nc.dram_tensor("out_p", [256, L], f16, kind="ExternalOutput")

    dma_rr = None

    with tile.TileContext(nc) as tc:
        with tc.tile_pool(name="const", bufs=1) as cpool, \
             tc.tile_pool(name="res", bufs=1) as rpool:
            convw = cpool.tile([128, M_TILES * K], f32)
            convb = cpool.tile([128, M_TILES], f32)
            dtb = cpool.tile([128, M_TILES], f32)
            arate = cpool.tile([128, M_TILES * N], f32)
            ident = cpool.tile([128, 128], f16)
            diagdp = cpool.tile([128, M_TILES * 128], f16)
            xpall = cpool.tile([128, M_TILES * 96], f16)
            dtwall = cpool.tile([DTR, M_TILES * 128], f16)
            mf = cpool.tile([128, 1], f32)
            mb = cpool.tile([128, 1], f32)
            engs = [nc.sync, nc.gpsimd, nc.scalar]

            xc16 = rpool.tile([128, M_TILES * L], f16)
            sz16 = rpool.tile([128, M_TILES * L], f16)
            g16 = rpool.tile([128, M_TILES * L], f16)
            delta16 = rpool.tile([128, M_TILES * L], f16)
            bca = rpool.tile([128, N * L], f16)
            bcc = rpool.tile([128, N * L], f16)
            dtrows = rpool.tile([DTR, L], f16)

            # ---------- Phase A: in_proj + conv + silu + x_proj accum ----------
            with tc.tile_pool(name="pa", bufs=1) as pap, \
                 tc.tile_pool(name="paw", bufs=4) as pwp, \
                 tc.tile_pool(name="pcv", bufs=2) as pcv, \
                 tc.tile_pool(name="psA1", bufs=2, space="PSUM") as psA1, \
                 tc.tile_pool(name="psA2", bufs=1, space="PSUM") as psA2, \
                 tc.tile_pool(name="psB", bufs=1, space="PSUM") as psB:
                xT = pap.tile([128, M_TILES * (3 + L)], f16)
                # single strided DMA for the whole input (keeps sync/scalar
                # queues free so m0's weights land first)
                nc.gpsimd.dma_start(
                    xT[:].rearrange("p (k c) -> p k c", c=3 + L),
                    xT_d[:].rearrange("(k p) c -> p k c", p=128))
                # consts are consumed late; issue them behind the hot DMAs
                nc.sync.dma_start(
                    xpall[:].rearrange("p (m f) -> p m f", f=96),
                    xpT_d[:].rearrange("(m p) f -> p m f", p=128))
                for i, (t_, d_) in enumerate((
                        (convw, convw_d), (convb, convb_d), (dtb, dtb_d),
                        (arate, arate_d), (ident, ident_d), (diagdp, diagdp_d),
                        (dtwall, dtwT_d), (mf, mf_d), (mb, mb_d))):
                    engs[i % 3].dma_start(t_[:], d_[:])
                ps_dbl = psB.tile([96, L], f32)
                for m in range(M_TILES):
                    wxi = pwp.tile([128, EH], f16, tag="wxi")
                    wz = pwp.tile([128, EH], f16, tag="wz")
                    nc.sync.dma_start(wxi[:, 0:512], wxiT_d[:, m * EH:m * EH + 512])
                    nc.gpsimd.dma_start(wxi[:, 512:EH], wxiT_d[:, m * EH + 512:(m + 1) * EH])
                    nc.scalar.dma_start(wz[:, 0:512], wzT_d[:, m * EH:m * EH + 512])
                    nc.sync.dma_start(wz[:, 512:EH], wzT_d[:, m * EH + 512:(m + 1) * EH])
                    ps_xi = psA1.tile([128, L], f32, tag="xi")
                    ps_z = psA2.tile([128, L], f32, tag="z")
                    for kt in range(M_TILES):
                        xk = xT[:, kt * (3 + L):(kt + 1) * (3 + L)]
                        for h in range(2):
                            nc.tensor.matmul(ps_xi[:, h * 512:(h + 1) * 512],
                                             wxi[:, kt * 128:(kt + 1) * 128],
                                             xk[:, 3 + h * 512: 3 + (h + 1) * 512],
                                             start=(kt == 0), stop=(kt == M_TILES - 1))
                    for kt in range(M_TILES):
                        xk = xT[:, kt * (3 + L):(kt + 1) * (3 + L)]
                        for h in range(2):
                            nc.tensor.matmul(ps_z[:, h * 512:(h + 1) * 512],
                                             wz[:, kt * 128:(kt + 1) * 128],
                                             xk[:, 3 + h * 512: 3 + (h + 1) * 512],
                                             start=(kt == 0), stop=(kt == M_TILES - 1))
                    # conv: f16 padded copy, 4 taps as tensor_scalar muls + adds
                    xi16 = pcv.tile([128, 3 + L], f16, tag="xi16")
                    if m < 2:
                        nc.vector.memset(xi16[:, 0:3], 0.0)
                    nc.scalar.activation(xi16[:, 3:3 + L], ps_xi[:], Act.Copy)
                    tp = pcv.tile([128, 4 * L], f16, tag="taps")
                    for k in range(K):
                        nc.vector.tensor_scalar_mul(tp[:, k * L:(k + 1) * L],
                                                    xi16[:, k:k + L],
                                                    convw[:, m * K + k:m * K + k + 1])
                    t01 = pcv.tile([128, L], f16, tag="t01")
                    t23 = pcv.tile([128, L], f16, tag="t23")
                    cacc = pcv.tile([128, L], f16, tag="cacc")
                    nc.vector.tensor_add(t01[:], tp[:, 0:L], tp[:, L:2 * L])
                    nc.vector.tensor_add(t23[:], tp[:, 2 * L:3 * L], tp[:, 3 * L:4 * L])
                    nc.vector.tensor_add(cacc[:], t01[:], t23[:])
                    nc.scalar.activation(xc16[:, m * L:(m + 1) * L], cacc[:],
                                         Act.Silu, bias=convb[:, m:m + 1])
                    nc.scalar.activation(sz16[:, m * L:(m + 1) * L], ps_z[:], Act.Silu)
                    for h in range(2):
                        nc.tensor.matmul(ps_dbl[:, h * 512:(h + 1) * 512],
                                         xpall[:, m * 96:(m + 1) * 96],
                                         xc16[:, m * L + h * 512: m * L + (h + 1) * 512],
                                         start=(m == 0), stop=(m == M_TILES - 1))
                dbl16 = pap.tile([96, L], f16)
                nc.vector.tensor_copy(dbl16[:], ps_dbl[:])
                nc.sync.dma_start(ar_in[:], dbl16[:])
                # dt rows first so softplus can start while B/C rows reduce
                nc.gpsimd.collective_compute(
                    "AllReduce", Alu.add,
                    replica_groups=[[0, 1], [2, 3], [4, 5], [6, 7]],
                    ins=[ar_in[0:DTR, :]], outs=[ar_out1[:]])
                nc.gpsimd.collective_compute(
                    "AllReduce", Alu.add,
                    replica_groups=[[0, 1], [2, 3], [4, 5], [6, 7]],
                    ins=[ar_in[DTR:96, :]], outs=[ar_out2[:]])
                nc.sync.dma_start(dtrows[:], ar_out1[:])
                nc.sync.dma_start(
                    bca[:].rearrange("p (n l) -> p n l", l=L),
                    ar_out2[None, 0:N, :].broadcast_to([128, N, L]))
                nc.gpsimd.dma_start(
                    bcc[:].rearrange("p (n l) -> p n l", l=L),
                    ar_out2[None, N:2 * N, :].broadcast_to([128, N, L]))

            # ---------- Phase C: softplus (bursts of 3), dA planes, scan ----------
            with tc.tile_pool(name="pee", bufs=3) as pee, \
                 tc.tile_pool(name="pc", bufs=2) as pcp, \
                 tc.tile_pool(name="psY", bufs=2, space="PSUM") as psY:
                bca3 = bca[:].rearrange("p (n l) -> p n l", l=L)
                bcc3 = bcc[:].rearrange("p (n l) -> p n l", l=L)

                def cmain(m):
                    u16 = pcp.tile([128, L], f16, tag="u16")
                    nc.vector.tensor_mul(u16[:], delta16[:, m * L:(m + 1) * L],
                                         xc16[:, m * L:(m + 1) * L])
                    ps_y = psY.tile([128, L], f32, tag="y")
                    for g in range(NG):
                        dA = pcp.tile([128, NPB * PL], f16, tag="dA")
                        for j in range(NPB):
                            n = g * NPB + j
                            nc.scalar.activation(dA[:, j * PL:j * PL + L],
                                                 delta16[:, m * L:(m + 1) * L],
                                                 Act.Exp,
                                                 scale=arate[:, m * N + n:m * N + n + 1])
                        dA3 = dA[:].rearrange("p (n l) -> p n l", l=PL)
                        dBu = pcp.tile([128, NPB * PL], f16, tag="dBu")
                        dBu3 = dBu[:].rearrange("p (n l) -> p n l", l=PL)
                        if m == 0 and g < 2:
                            # gap columns stay 0 across slot reuse (2 slots/tag)
                            nc.vector.memset(dA3[:, :, L:PL], 0.0)
                            nc.vector.memset(dBu3[:, :, L:PL], 0.0)
                        nc.vector.tensor_mul(
                            dBu3[:, :, 0:L],
                            u16[:, None, :].broadcast_to([128, NPB, L]),
                            bca3[:, g * NPB:(g + 1) * NPB, :])
                        h4 = pcp.tile([128, NPB * PL], f16, tag="h4")
                        nc.vector.tensor_tensor_scan(h4[:], dA[:], dBu[:], 0.0,
                                                     Alu.mult, Alu.add)
                        h43 = h4[:].rearrange("p (n l) -> p n l", l=PL)
                        prod = pcp.tile([128, NPB * PL], f16, tag="dBu")
                        prod3 = prod[:].rearrange("p (n l) -> p n l", l=PL)
                        nc.vector.tensor_mul(prod3[:, :, 0:L], h43[:, :, 0:L],
                                             bcc3[:, g * NPB:(g + 1) * NPB, :])
                        for j in range(NPB):
                            for h in range(2):
                                nc.tensor.matmul(
                                    ps_y[:, h * 512:(h + 1) * 512], ident[:],
                                    prod[:, j * PL + h * 512: j * PL + h * 512 + 512],
                                    start=(g == 0 and j == 0), stop=False)
                    # dp * xc skip-connection rides a diagonal matmul
                    for h in range(2):
                        nc.tensor.matmul(ps_y[:, h * 512:(h + 1) * 512],
                                         diagdp[:, m * 128:(m + 1) * 128],
                                         xc16[:, m * L + h * 512: m * L + (h + 1) * 512],
                                         start=False, stop=(h == 1))
                    y16s = pcp.tile([128, L], f16, tag="y16s")
                    nc.scalar.activation(y16s[:], ps_y[:], Act.Copy)
                    nc.vector.tensor_mul(g16[:, m * L:(m + 1) * L], y16s[:],
                                         sz16[:, m * L:(m + 1) * L])

                with tc.tile_pool(name="psP", bufs=2, space="PSUM") as psP:
                    # softplus in bursts of 3 m-tiles (Exp x3 then Ln x3) so the
                    # scalar engine switches act tables ~6x total, and C-main
                    # m=0 unblocks after the first burst.
                    for m0 in range(0, M_TILES, 3):
                        ms = range(m0, min(m0 + 3, M_TILES))
                        ees = {}
                        for m in ms:
                            ps_dt = psP.tile([128, L], f32, tag="dt")
                            for h in range(2):
                                nc.tensor.matmul(ps_dt[:, h * 512:(h + 1) * 512],
                                                 dtwall[:, m * 128:(m + 1) * 128],
                                                 dtrows[:, h * 512:(h + 1) * 512],
                                                 start=True, stop=True)
                            ee = pee.tile([128, L], f32, tag="ee")
                            nc.scalar.activation(ee[:], ps_dt[:],
                                                 Act.Exp, bias=dtb[:, m:m + 1])
                            ees[m] = ee
                        for m in ms:
                            nc.scalar.activation(delta16[:, m * L:(m + 1) * L],
                                                 ees[m][:], Act.Ln, bias=1.0)
                    for m in range(M_TILES):
                        cmain(m)

            # ---------- Phase D: out_proj + flip-combine + ReduceScatter ----------
            with tc.tile_pool(name="pd", bufs=2) as pdp, \
                 tc.tile_pool(name="psD", bufs=2, space="PSUM") as psD:
                for dm in range(M_TILES):
                    wo = pdp.tile([128, D], f16, tag="wo")
                    nc.sync.dma_start(wo[:, 0:512], woT_d[:, dm * D:dm * D + 512])
                    nc.gpsimd.dma_start(wo[:, 512:D], woT_d[:, dm * D + 512:(dm + 1) * D])
                    ps_o = psD.tile([128, L], f32, tag="o")
                    for m in range(M_TILES):
                        for h in range(2):
                            nc.tensor.matmul(ps_o[:, h * 512:(h + 1) * 512],
                                             wo[:, m * 128:(m + 1) * 128],
                                             g16[:, m * L + h * 512: m * L + (h + 1) * 512],
                                             start=(m == 0), stop=(m == M_TILES - 1))
                    t1 = pdp.tile([128, L], f16, tag="t1")
                    r1 = pdp.tile([128, L], f16, tag="r1")
                    ocs = pdp.tile([128, L], f16, tag="ocs")
                    nc.vector.tensor_scalar_mul(t1[:], ps_o[:], mf[:, 0:1])
                    nc.vector.tensor_scalar_mul(r1[:], ps_o[:, ::-1], mb[:, 0:1])
                    nc.vector.tensor_add(ocs[:], t1[:], r1[:])
                    nc.sync.dma_start(oc_in[dm * 128:(dm + 1) * 128, :], ocs[:])
                    if dm % 2 == 1:
                        # ReduceScatter: group-rank ci gets a contiguous 64-row
                        # shard of each 256-row chunk; host stitches shards.
                        ch = dm // 2
                        nc.gpsimd.collective_compute(
                            "ReduceScatter", Alu.add,
                            replica_groups=[[0, 1, 2, 3], [4, 5, 6, 7]],
                            ins=[oc_in[ch * 256:(ch + 1) * 256, :]],
                            outs=[oc_out[ch * 64:(ch + 1) * 64, :]])
                        nc.sync.dma_start(out_d[ch * 64:(ch + 1) * 64, :],
                                          oc_out[ch * 64:(ch + 1) * 64, :])

    nc.compile()
    return nc


def _host_prep(inputs):
    """Build the 8 per-core input maps from the full problem inputs."""
    x = np.asarray(inputs["x"], np.float32)
    merge_w = np.asarray(inputs["merge_w"], np.float32)
    in_maps = []
    for b in range(B):
        for di, pre in enumerate(("fwd", "bwd")):
            p = {k: np.asarray(inputs[f"{pre}_{k}"], np.float32)
                 for k in ("in_proj", "conv_w", "conv_b", "x_proj", "dt_w",
                           "dt_b", "A_log", "D", "out_proj")}
            xb = x[b]
            if di == 1:
                xb = xb[::-1]
            xTp = np.concatenate([np.zeros((D, 3), np.float32), xb.T], axis=1)
            A = -np.exp(p["A_log"])                       # (E, N)
            W = merge_w[:, di * D:(di + 1) * D] @ p["out_proj"]   # (D, E)
            def pack_lhsT(wT):
                # (D, EH) -> [p, m*1024 + kt*128 + e']
                return np.ascontiguousarray(
                    wT.reshape(M_TILES, 128, M_TILES, 128).transpose(1, 2, 0, 3)
                    .reshape(128, M_TILES * EH))

            for half in range(2):
                sl = slice(half * EH, (half + 1) * EH)
                wxiT = pack_lhsT(p["in_proj"][:E][sl].T)
                wzT = pack_lhsT(p["in_proj"][E:][sl].T)
                convw = p["conv_w"][sl].reshape(M_TILES, 128, K).transpose(1, 0, 2).reshape(128, M_TILES * K)
                convb = p["conv_b"][sl].reshape(M_TILES, 128).T
                xpT = p["x_proj"][:, sl].T                # (EH, 96)
                dtwT = p["dt_w"][sl].T                    # (DTR, EH)
                dtb = p["dt_b"][sl].reshape(M_TILES, 128).T
                arate = A[sl].reshape(M_TILES, 128, N).transpose(1, 0, 2).reshape(128, M_TILES * N)
                dp128 = p["D"][sl].reshape(M_TILES, 128).T    # [128, M]
                diagdp = np.zeros((128, M_TILES * 128), np.float32)
                for m in range(M_TILES):
                    diagdp[np.arange(128), m * 128 + np.arange(128)] = dp128[:, m]
                woT = pack_lhsT(W[:, sl].T)               # (EH, D) pre-tiled
                fwd = (di == 0)
                in_maps.append({
                    "xT": xTp.astype(np.float16),
                    "wxiT": wxiT.astype(np.float16),
                    "wzT": wzT.astype(np.float16),
                    "convw": np.ascontiguousarray(convw, np.float32),
                    "convb": np.ascontiguousarray(convb, np.float32),
                    "xpT": xpT.astype(np.float16),
                    "dtwT": np.ascontiguousarray(dtwT).astype(np.float16),
                    "dtb": np.ascontiguousarray(dtb, np.float32),
                    "arate": np.ascontiguousarray(arate, np.float32),
                    "ident": np.eye(128, dtype=np.float16),
                    "diagdp": diagdp.astype(np.float16),
                    "woT": woT.astype(np.float16),
                    "mf": np.full((128, 1), 1.0 if fwd else 0.0, np.float32),
                    "mb": np.full((128, 1), 0.0 if fwd else 1.0, np.float32),
                })
    return in_maps


def _ensure_neuron_platform():
    """If a caller pinned jax to cpu, re-point it at the neuron/axon PJRT
    platform so run_bass_kernel_spmd sees the 8 NeuronCores."""
    import jax
    try:
        if len(jax.devices()) >= 8 and jax.devices()[0].platform != "cpu":
            return
    except Exception:
        pass
    for plat in ("axon", "neuron"):
        try:
            jax.config.update("jax_platforms", plat)
            if len(jax.devices()) >= 8:
                return
        except Exception:
            continue


def kernel(**inputs):
    _ensure_neuron_platform()
    from concourse.bass_utils import run_bass_kernel_spmd
    if "nc" not in _nc_cache:
        _nc_cache["nc"] = _build_nc()
    nc = _nc_cache["nc"]
    in_maps = _host_prep(inputs)
    res = run_bass_kernel_spmd(nc, in_maps, core_ids=list(range(8)))
    _nc_cache["last_results"] = res
    # Stitch ReduceScatter shards: 4 chunks of 256 d-rows; within chunk ch,
    # group-rank ci holds rows [ch*256 + ci*64 : +64] at out_p[ch*64:(ch+1)*64].
    out = np.zeros((B, L, D), np.float32)
    for b in range(B):
        od = np.zeros((D, L), np.float32)
        for ci in range(4):
            shard = res.results[4 * b + ci]["out_p"].astype(np.float32)
            for ch in range(4):
                od[ch * 256 + ci * 64: ch * 256 + (ci + 1) * 64] = \
                    shard[ch * 64:(ch + 1) * 64]
        out[b] = od.T
    return out
